# revision 21
# baseline (speedup 1.0000x reference)
"""Trainium2 Bass kernel for nn_CLM_23038204575917 (dense transformer CLM).

Sharding: DP=2 over batch x TP=4 within batch group.
  core c (0..7): batch g = c//4, TP rank r = c%4.
  - attention heads: 4 per core (of 16), head-dim 64 -> 256 attn channels
  - FFN hidden: 1024 per core (of 4096)
  - lm_head vocab: 8000 per core (of 32000), padded to 8064
Activations kept transposed [E, tok] in bf16; LN gamma/beta folded into
weights host-side; softmax without max-subtraction (scores tiny), causal
mask applied multiplicatively after exp; softmax denom via ones-column
in V; normalization via DVE divide (2x mode) instead of the slow serial
InstReciprocal; all row-broadcasts ride bf16 ones-matmuls (fp32 matmuls
run LOW/HIGH double passes - avoided).

v2 layout/scheduling:
  - layer emitted as 4 chunk-passes (attn c0, attn c1, ffn c0, ffn c1),
    each ending in its AllReduce, so every AR overlaps the next pass's
    AR-independent PE work (the static per-engine instruction order
    stalls head-of-line otherwise).
  - x/h/dsb/ds are merged [128, 8*CH] tiles; AR bounce is ONE DMA each
    way (was 8), readback on the scalar queue, bounce-in on gpsimd.
  - weights DMA'd as single merged [128, 8*X] tiles per matrix.
  - lm_head: emb pre-packed host-side to [8064, 1024] so each vocab tile
    is one 256KB DMA; ci-major loop so chunk-0 logits hide the last AR;
    logits written bf16 (upcast host-side).
"""

import contextlib
import ctypes
import sys
import types

import numpy as np

sys.path.insert(0, "/opt/trn_rl_repo")

import ml_dtypes

bf16 = ml_dtypes.bfloat16

# ---------------------------------------------------------------- ntff hook
# Allows run_bass_kernel_spmd(trace=True) / BASS_TRACE=1 to profile through
# the axon PJRT plugin even though the image's antenv lacks axon_hooks.
if "antenv.axon_hooks" not in sys.modules:
    def _ntff_profile_via_ctypes(so_path):
        try:
            lib = ctypes.CDLL(so_path)
        except OSError:
            return None
        if not hasattr(lib, "axon_start_nrt_profile"):
            return None
        lib.axon_start_nrt_profile.argtypes = [ctypes.POINTER(ctypes.c_int64), ctypes.c_size_t]
        lib.axon_start_nrt_profile.restype = ctypes.c_int64
        lib.axon_stop_nrt_profile.argtypes = [ctypes.c_char_p]
        lib.axon_stop_nrt_profile.restype = ctypes.c_int64

        @contextlib.contextmanager
        def _hook(output_dir, device_ids):
            import jax
            jax.devices()
            if device_ids:
                ids = (ctypes.c_int64 * len(device_ids))(*device_ids)
                rc = lib.axon_start_nrt_profile(ids, len(device_ids))
            else:
                rc = lib.axon_start_nrt_profile(None, 0)
            if rc != 0:
                raise RuntimeError(f"axon_start_nrt_profile rc={rc}")
            try:
                yield
            finally:
                n = lib.axon_stop_nrt_profile(str(output_dir).encode())
                print(f"ntff profile: {n} file(s) -> {output_dir}", file=sys.stderr)

        return _hook

    _mod = types.ModuleType("antenv.axon_hooks")
    _mod._hook = _ntff_profile_via_ctypes("/opt/axon/libaxon_pjrt.so")
    _mod.get_axon_ntff_profile_hook = lambda: _mod._hook
    _mod.set_axon_ntff_profile_hook = lambda h: setattr(_mod, "_hook", h)
    sys.modules["antenv.axon_hooks"] = _mod

import concourse.bass as bass
import concourse.tile as tile
from concourse import mybir
from concourse.bass_utils import run_bass_kernel_spmd

DT = mybir.dt
AF = mybir.ActivationFunctionType
ALU = mybir.AluOpType

# Model dims
V, T, E, H, L, FFD = 32000, 1024, 1024, 16, 4, 4096
HD = 64
NCORES = 8
TPD = 4                  # tensor-parallel degree within a batch group
HC = H // TPD            # heads per core = 4
C = HC * HD              # attn channels per core = 256
F = FFD // TPD           # ffn hidden per core = 1024
VC = V // TPD            # vocab slice per core = 8000
VCP = 8064               # padded to 63*128
ET = E // 128            # 8 e-tiles
NCH = 2                  # token chunks of 512
CH = 512
VW = 260                 # HC * 65 v columns (64 dims + ones col per head)
GROUPS = [[0, 1, 2, 3], [4, 5, 6, 7]]


def _split_sync_waits(nc, max_waits=1):
    """This env's walrus rejects >1 sem-wait per instruction; move excess
    waits onto same-engine NoOps inserted just before."""
    for fn in nc.m.functions:
        for bb in fn.blocks:
            new_list = []
            for ins in bb.instructions:
                si = ins.sync_info
                if si is not None and si.on_wait and len(si.on_wait) > max_waits:
                    waits = list(si.on_wait)
                    extra, keep = waits[:-max_waits], waits[-max_waits:]
                    for k in range(0, len(extra), max_waits):
                        nop = mybir.InstNoOp(name=f"{ins.name}-ws{k}", ins=[], outs=[])
                        nop.engine = ins.engine
                        nop.sync_info = mybir.SyncInfo(
                            on_wait=extra[k:k + max_waits], on_update=[])
                        new_list.append(nop)
                    si.on_wait = keep
                new_list.append(ins)
            bb.instructions[:] = new_list


def _build_program():
    nc = bass.Bass()
    inp = {}

    def din(name, shape, dt=DT.bfloat16):
        inp[name] = nc.dram_tensor(name, list(shape), dt, kind="ExternalInput")
        return inp[name]

    x0A_d = din("x0A", (NCH, 128, ET * CH))
    embA_d = din("embA", (VCP, ET * 128))
    masks_d = din("masks", (4, 128, CH))
    invE_d = din("invE", (128, 1))
    ones128_d = din("ones128", (1, 128))
    ones64_d = din("ones64", (1, 64))
    for l in range(L):
        din(f"wqA{l}", (128, ET * C)); din(f"wkA{l}", (128, ET * C))
        din(f"wvA{l}", (128, ET * VW))
        din(f"bqk{l}", (128, 4), DT.float32)       # cols: bq0,bq1,bk0,bk1
        din(f"bvrow{l}", (1, VW))
        din(f"wpA{l}", (128, 2 * E))
        din(f"w1A{l}", (128, ET * F))
        din(f"w2A{l}", (128, ET * E))
        din(f"bf2{l}", (128, 2 * ET), DT.float32)  # cols 0..7 bf, 8..15 b2q
    out_d = nc.dram_tensor("logitsT", [VCP, T], DT.bfloat16, kind="ExternalOutput")

    with tile.TileContext(nc) as tc, contextlib.ExitStack() as ctx:
        cpool = ctx.enter_context(tc.tile_pool(name="const", bufs=1))
        xpool = ctx.enter_context(tc.tile_pool(name="x", bufs=1))
        hpool = ctx.enter_context(tc.tile_pool(name="h", bufs=1))
        wpool = ctx.enter_context(tc.tile_pool(name="w", bufs=1))
        wbig = ctx.enter_context(tc.tile_pool(name="wbig", bufs=1))
        qkv = ctx.enter_context(tc.tile_pool(name="qkv", bufs=1))
        wexpp = ctx.enter_context(tc.tile_pool(name="wexp", bufs=1))
        opool = ctx.enter_context(tc.tile_pool(name="o", bufs=1))
        dpool = ctx.enter_context(tc.tile_pool(name="d", bufs=1))
        rowp = ctx.enter_context(tc.tile_pool(name="rows", bufs=1))
        lsbp = ctx.enter_context(tc.tile_pool(name="lsb", bufs=3))
        embp = ctx.enter_context(tc.tile_pool(name="emb", bufs=2))
        dram = ctx.enter_context(tc.tile_pool(name="dram", bufs=1, space="DRAM"))
        ps_mm = ctx.enter_context(tc.tile_pool(name="psmm", bufs=2, space="PSUM"))
        ps_w = ctx.enter_context(tc.tile_pool(name="psw", bufs=3, space="PSUM"))
        ps_o = ctx.enter_context(tc.tile_pool(name="pso", bufs=2, space="PSUM"))
        ps_s = ctx.enter_context(tc.tile_pool(name="pss", bufs=1, space="PSUM"))

        # ---- constants
        invE = cpool.tile([128, 1], DT.bfloat16, tag="invE", name="invE")
        nc.sync.dma_start(invE[:], invE_d[:])
        ones128 = cpool.tile([1, 128], DT.bfloat16, tag="ones128", name="ones128")
        nc.sync.dma_start(ones128[:], ones128_d[:])
        ones64 = cpool.tile([1, 64], DT.bfloat16, tag="ones64", name="ones64")
        nc.sync.dma_start(ones64[:], ones64_d[:])
        eps128 = cpool.tile([128, 1], DT.float32, tag="eps128", name="eps128")
        nc.gpsimd.memset(eps128[:], 1e-5)
        masks = [cpool.tile([128, CH], DT.bfloat16, tag=f"mask{j}", name=f"mask{j}") for j in range(4)]
        for j in range(4):
            nc.sync.dma_start(masks[j][:], masks_d[j])

        # ---- residual, merged per-chunk tiles x[ci] = [128, 8*CH]
        x = [xpool.tile([128, ET * CH], DT.bfloat16, tag=f"x{ci}", name=f"x{ci}")
             for ci in range(NCH)]
        for ci in range(NCH):
            nc.sync.dma_start(x[ci][:], x0A_d[ci])

        def esl(et):
            return slice(et * CH, (et + 1) * CH)

        def ln_chunk(hdst, ci):
            """hdst[:, et*CH:(et+1)*CH] = (x - mu) / sd for token chunk ci.

            Tile sums over the 8 e-tiles ride DVE tree-adds (PE does just 2
            reduction matmuls instead of 16)."""
            xc = x[ci]
            mom = ps_s.tile([33, CH], DT.float32, tag="mom", name="mom")
            mu_ps, m2_ps = mom[0:1, :], mom[32:33, :]
            for et in range(ET):
                nc.tensor.matmul(mu_ps, invE[:], xc[:, esl(et)],
                                 start=(et == 0), stop=(et == ET - 1))
            for et in range(ET):
                xsq = hpool.tile([128, CH], DT.bfloat16, tag="xsq", bufs=3, name="xsq")
                nc.vector.tensor_tensor(xsq[:], xc[:, esl(et)], xc[:, esl(et)], op=ALU.mult)
                nc.tensor.matmul(m2_ps, invE[:], xsq[:],
                                 start=(et == 0), stop=(et == ET - 1))
            mu = rowp.tile([1, CH], DT.float32, tag="mu_sb", name="mu_sb")
            nc.scalar.activation(mu[:], mu_ps, AF.Identity)
            mu2 = rowp.tile([1, CH], DT.float32, tag="mu2_sb", name="mu2_sb")
            nc.vector.tensor_tensor(mu2[:], mu[:], mu[:], op=ALU.mult)
            var = rowp.tile([1, CH], DT.float32, tag="var_sb", name="var_sb")
            nc.vector.tensor_tensor(var[:], m2_ps, mu2[:], op=ALU.subtract)
            sd = rowp.tile([1, CH], DT.float32, tag="sd_sb", name="sd_sb")
            nc.scalar.activation(sd[:], var[:], AF.Sqrt, bias=eps128[0:1, :])
            a32 = rowp.tile([1, CH], DT.float32, tag="a32", name="a32")
            nc.vector.reciprocal(a32[:], sd[:])
            a16 = rowp.tile([1, CH], DT.bfloat16, tag="a16", name="a16")
            nc.scalar.activation(a16[:], a32[:], AF.Identity)
            c16 = rowp.tile([1, CH], DT.bfloat16, tag="c16", name="c16")
            nc.vector.tensor_tensor(c16[:], mu[:], a32[:], op=ALU.mult)
            ab_ps = ps_w.tile([128, CH], DT.float32, tag="w", name="ab")
            nc.tensor.matmul(ab_ps[:], ones128[:], a16[:], start=True, stop=True)
            cb_ps = ps_w.tile([128, CH], DT.float32, tag="w", name="cb")
            nc.tensor.matmul(cb_ps[:], ones128[:], c16[:], start=True, stop=True)
            a_b = rowp.tile([128, CH], DT.bfloat16, tag="ab_sb", bufs=2, name="ab_sb")
            nc.scalar.activation(a_b[:], ab_ps[:], AF.Identity)
            c_b = rowp.tile([128, CH], DT.bfloat16, tag="cb_sb", bufs=2, name="cb_sb")
            nc.scalar.activation(c_b[:], cb_ps[:], AF.Identity)
            for et in range(ET):
                tmp = hpool.tile([128, CH], DT.bfloat16, tag="lnt", name="lnt")
                nc.vector.tensor_tensor(tmp[:], xc[:, esl(et)], a_b[:], op=ALU.mult)
                nc.vector.tensor_tensor(hdst[:, esl(et)], tmp[:], c_b[:], op=ALU.subtract)

        def ar_block(dsb, l, phase, ci):
            """bounce dsb -> AllReduce(group of 4) -> residual add into x[ci]."""
            dloc = dram.tile([128, ET * CH], DT.bfloat16,
                             tag=f"dloc_{phase}{l}_{ci}", name="dloc")
            dred = dram.tile([128, ET * CH], DT.bfloat16,
                             tag=f"dred_{phase}{l}_{ci}", name="dred")
            nc.gpsimd.dma_start(dloc[:], dsb[:])
            nc.gpsimd.collective_compute(
                "AllReduce", ALU.add, replica_groups=GROUPS,
                ins=[dloc.opt()], outs=[dred.opt()])
            ds = dpool.tile([128, ET * CH], DT.bfloat16, tag="dsA", bufs=2, name="dsA")
            nc.scalar.dma_start(ds[:], dred[:])
            for p in range(4):
                psl2 = slice(2 * p * CH, (2 * p + 2) * CH)
                nc.vector.tensor_tensor(x[ci][:, psl2], x[ci][:, psl2],
                                        ds[:, psl2], op=ALU.add)

        # persistent per-layer qkv tiles
        for l in range(L):
            # ---- layer weights to SBUF (merged single DMAs)
            wq = wpool.tile([128, ET * C], DT.bfloat16, tag="wq", name="wq")
            wk = wpool.tile([128, ET * C], DT.bfloat16, tag="wk", name="wk")
            wv = wpool.tile([128, ET * VW], DT.bfloat16, tag="wv", name="wv")
            wp = wpool.tile([128, 2 * E], DT.bfloat16, tag="wp", name="wp")
            bqk = wpool.tile([128, 4], DT.float32, tag="bqk", name="bqk")
            bvrow = wpool.tile([1, VW], DT.bfloat16, tag="bvrow", name="bvrow")
            nc.sync.dma_start(wq[:], inp[f"wqA{l}"][:])
            nc.sync.dma_start(wk[:], inp[f"wkA{l}"][:])
            nc.sync.dma_start(wv[:], inp[f"wvA{l}"][:])
            nc.sync.dma_start(wp[:], inp[f"wpA{l}"][:])
            nc.sync.dma_start(bqk[:], inp[f"bqk{l}"][:])
            nc.sync.dma_start(bvrow[:], inp[f"bvrow{l}"][:])
            w1 = wbig.tile([128, ET * F], DT.bfloat16, tag="w1", name="w1")
            w2 = wbig.tile([128, ET * E], DT.bfloat16, tag="w2", name="w2")
            bf2 = wpool.tile([128, 2 * ET], DT.float32, tag="bf2", name="bf2")
            nc.sync.dma_start(w1[:], inp[f"w1A{l}"][:])
            nc.sync.dma_start(w2[:], inp[f"w2A{l}"][:])
            nc.sync.dma_start(bf2[:], inp[f"bf2{l}"][:])

            qT = [[qkv.tile([128, CH], DT.bfloat16, tag=f"qT{ct}_{ci}", name=f"qT{ct}_{ci}")
                   for ci in range(NCH)] for ct in range(2)]
            kT = [[qkv.tile([128, CH], DT.bfloat16, tag=f"kT{ct}_{ci}", name=f"kT{ct}_{ci}")
                   for ci in range(NCH)] for ct in range(2)]
            v = [qkv.tile([128, VW], DT.bfloat16, tag=f"v{tt}", name=f"v{tt}")
                 for tt in range(8)]

            # ======== attention passes, one chunk at a time ========
            for ci in range(NCH):
                h = hpool.tile([128, ET * CH], DT.bfloat16, tag=f"h{ci}", name=f"h{ci}")
                ln_chunk(h, ci)

                # Q, K projections for this chunk
                for ct in range(2):
                    pq = ps_mm.tile([128, CH], DT.float32, tag="mm", name="mm")
                    for et in range(ET):
                        nc.tensor.matmul(pq[:], wq[:, et * C + ct * 128: et * C + (ct + 1) * 128],
                                         h[:, esl(et)], start=(et == 0), stop=(et == ET - 1))
                    nc.scalar.activation(qT[ct][ci][:], pq[:], AF.Identity,
                                         bias=bqk[:, ct:ct + 1])
                    pk = ps_mm.tile([128, CH], DT.float32, tag="mm", name="mm")
                    for et in range(ET):
                        nc.tensor.matmul(pk[:], wk[:, et * C + ct * 128: et * C + (ct + 1) * 128],
                                         h[:, esl(et)], start=(et == 0), stop=(et == ET - 1))
                    nc.scalar.activation(kT[ct][ci][:], pk[:], AF.Identity,
                                         bias=bqk[:, 2 + ct:3 + ct])

                # V (token-major, with ones column) for this chunk's 4 tiles
                for tt in range(4 * ci, 4 * ci + 4):
                    lsl = slice((tt % 4) * 128, (tt % 4) * 128 + 128)
                    pv = ps_mm.tile([128, VW], DT.float32, tag="mm", name="mm")
                    for et in range(ET):
                        nc.tensor.matmul(pv[:], h[:, et * CH + (tt % 4) * 128: et * CH + (tt % 4) * 128 + 128],
                                         wv[:, et * VW:(et + 1) * VW],
                                         start=(et == 0), stop=False)
                    nc.tensor.matmul(pv[:], ones128[:], bvrow[:], start=False, stop=True)
                    nc.vector.tensor_copy(v[tt][:], pv[:])

                # attention for this chunk; head-pairs packed on PE row groups
                o2 = [opool.tile([128, CH], DT.bfloat16, tag=f"o2_{hp}_{ci}", name=f"o2_{hp}_{ci}")
                      for hp in range(2)]
                nkt = 4 * ci + 4
                for hp in range(2):
                    ct = hp
                    opsA = ps_o.tile([65, CH], DT.float32, tag="o", name="oA")
                    opsB = ps_o.tile([65, CH], DT.float32, tag="o", name="oB")
                    for kt in range(nkt):
                        j = kt - 4 * ci
                        wexs = []
                        for sub in range(2):
                            psl = slice(64 * sub, 64 * sub + 64)
                            kcj, klo = kt // 4, (kt % 4) * 128
                            pw = ps_w.tile([128, CH], DT.float32, tag="w", name="w")
                            nc.tensor.matmul(pw[:], kT[ct][kcj][psl, klo:klo + 128],
                                             qT[ct][ci][psl, :], start=True, stop=True)
                            wex = wexpp.tile([128, CH], DT.bfloat16, tag="wx", bufs=6,
                                             name=f"wx{kt}_{sub}")
                            if j >= 0:
                                tmp = wexpp.tile([128, CH], DT.bfloat16, tag="wxt",
                                                 bufs=2, name="wxt")
                                nc.scalar.activation(tmp[:], pw[:], AF.Exp, scale=0.125)
                                nc.vector.tensor_tensor(wex[:], tmp[:], masks[j][:],
                                                        op=ALU.mult)
                            else:
                                nc.scalar.activation(wex[:], pw[:], AF.Exp, scale=0.125)
                            wexs.append(wex)
                        hA, hB = 2 * hp, 2 * hp + 1
                        nc.tensor.matmul(opsA[:], v[kt][:, hA * 65:(hA + 1) * 65], wexs[0][:],
                                         start=(kt == 0), stop=(kt == nkt - 1))
                        nc.tensor.matmul(opsB[:], v[kt][:, hB * 65:(hB + 1) * 65], wexs[1][:],
                                         start=(kt == 0), stop=(kt == nkt - 1))
                    s32 = rowp.tile([33, CH], DT.float32, tag="s32", bufs=2, name="s32")
                    nc.scalar.activation(s32[0:1, :], opsA[64:65, :], AF.Identity)
                    nc.scalar.activation(s32[32:33, :], opsB[64:65, :], AF.Identity)
                    r32 = rowp.tile([33, CH], DT.float32, tag="r32", bufs=2, name="r32")
                    nc.vector.reciprocal(r32[:], s32[:])
                    r16 = [rowp.tile([1, CH], DT.bfloat16, tag=f"r16_{sub}", bufs=2,
                                     name=f"r16_{sub}") for sub in range(2)]
                    nc.scalar.activation(r16[0][:], r32[0:1, :], AF.Identity)
                    nc.scalar.activation(r16[1][:], r32[32:33, :], AF.Identity)
                    for sub, ops in ((0, opsA), (1, opsB)):
                        psl = slice(64 * sub, 64 * sub + 64)
                        rb = ps_w.tile([64, CH], DT.float32, tag="w", name="rb")
                        nc.tensor.matmul(rb[:], ones64[:], r16[sub][:],
                                         start=True, stop=True)
                        orw = rowp.tile([64, CH], DT.bfloat16, tag="orw", bufs=2, name="orw")
                        nc.scalar.activation(orw[:], ops[0:64, :], AF.Identity)
                        nc.vector.tensor_tensor(o2[hp][psl, :], orw[:], rb[:], op=ALU.mult)

                # proj for this chunk -> dsb -> AR -> residual
                dsb = dpool.tile([128, ET * CH], DT.bfloat16, tag="dsbA", bufs=2,
                                 name="dsbA")
                for et in range(ET):
                    pslE = slice(et * 128, (et + 1) * 128)
                    pd = ps_mm.tile([128, CH], DT.float32, tag="mm", name="mm")
                    nc.tensor.matmul(pd[:], wp[:, 0 * E + et * 128: 0 * E + (et + 1) * 128],
                                     o2[0][:], start=True, stop=False)
                    nc.tensor.matmul(pd[:], wp[:, 1 * E + et * 128: 1 * E + (et + 1) * 128],
                                     o2[1][:], start=False, stop=True)
                    nc.scalar.activation(dsb[:, esl(et)], pd[:], AF.Identity)
                ar_block(dsb, l, "a", ci)

            # ======== FFN passes, one chunk at a time ========
            for ci in range(NCH):
                h2 = hpool.tile([128, ET * CH], DT.bfloat16, tag=f"h{ci}", name=f"h2_{ci}")
                ln_chunk(h2, ci)
                ff = [dpool.tile([128, CH], DT.bfloat16, tag=f"ff{ft}", bufs=2,
                                 name=f"ff{ft}") for ft in range(ET)]
                for ft in range(ET):
                    pf = ps_mm.tile([128, CH], DT.float32, tag="mm", name="mm")
                    for et in range(ET):
                        nc.tensor.matmul(pf[:], w1[:, et * F + ft * 128: et * F + (ft + 1) * 128],
                                         h2[:, esl(et)], start=(et == 0), stop=(et == ET - 1))
                    nc.scalar.activation(ff[ft][:], pf[:], AF.Relu, bias=bf2[:, ft:ft + 1])
                dsb2 = dpool.tile([128, ET * CH], DT.bfloat16, tag="dsbA", bufs=2,
                                  name="dsb2")
                for et in range(ET):
                    pd = ps_mm.tile([128, CH], DT.float32, tag="mm", name="mm")
                    for ft in range(ET):
                        nc.tensor.matmul(pd[:], w2[:, ft * E + et * 128: ft * E + (et + 1) * 128],
                                         ff[ft][:], start=(ft == 0), stop=(ft == ET - 1))
                    nc.scalar.activation(dsb2[:, esl(et)], pd[:], AF.Identity,
                                         bias=bf2[:, ET + et:ET + et + 1])
                ar_block(dsb2, l, "f", ci)

        # ---- final LN + lm_head (ci-major so chunk 0 hides the last AR)
        for ci in range(NCH):
            hf = hpool.tile([128, ET * CH], DT.bfloat16, tag=f"h{ci}", name=f"hf{ci}")
            ln_chunk(hf, ci)
            sl = slice(ci * CH, (ci + 1) * CH)
            for vt in range(VCP // 128):
                vsl = slice(vt * 128, (vt + 1) * 128)
                emb = embp.tile([128, ET * 128], DT.bfloat16, tag="emb", name="emb")
                if vt % 2 == 0:
                    nc.sync.dma_start(emb[:], embA_d[vsl, :])
                else:
                    nc.scalar.dma_start(emb[:], embA_d[vsl, :])
                pool = ps_mm if vt % 2 == 0 else ps_w
                pl = pool.tile([128, CH], DT.float32,
                               tag=("mm" if vt % 2 == 0 else "w"), name="pl")
                for et in range(ET):
                    nc.tensor.matmul(pl[:], emb[:, et * 128:(et + 1) * 128],
                                     hf[:, esl(et)], start=(et == 0), stop=(et == ET - 1))
                lsb = lsbp.tile([128, CH], DT.bfloat16, tag="lsb", name="lsb")
                if vt % 2 == 0:
                    nc.scalar.activation(lsb[:], pl[:], AF.Identity)
                else:
                    nc.vector.tensor_copy(lsb[:], pl[:])
                nc.gpsimd.dma_start(out_d[vsl, sl], lsb[:])

    _split_sync_waits(nc)
    return nc


_NC = None


def _host_prep(inputs):
    """Fold LN params into weights, build per-core input maps."""
    f32 = np.float32

    def as_f32(vv):
        a = np.asarray(vv)
        return a if a.dtype in (np.int64, np.int32) else np.asarray(a, f32)

    g = {k: as_f32(vv) for k, vv in inputs.items()}
    idx = np.asarray(inputs["idx"])
    s = f32(E) ** -0.5

    mask_j = np.zeros((4, 128, CH), f32)
    q_idx = np.arange(CH)[None, :]
    k_idx = np.arange(128)[:, None]
    for j in range(4):
        mask_j[j] = (q_idx >= 128 * j + k_idx).astype(f32)

    def merge_et(mat, ncols):
        # [E, ncols] -> [128, ET*ncols] with et blocks along free axis
        return np.ascontiguousarray(
            mat.reshape(ET, 128, ncols).transpose(1, 0, 2).reshape(128, ET * ncols))

    per_layer = []
    for l in range(L):
        g1, b1v = g["ln1_g"][l], g["ln1_b"][l]
        g2, b2v = g["ln2_g"][l], g["ln2_b"][l]
        lay = []
        for r in range(TPD):
            csl = slice(C * r, C * (r + 1))
            fsl = slice(F * r, F * (r + 1))
            Wq_r, Wk_r, Wv_r = g["Wq"][l][csl], g["Wk"][l][csl], g["Wv"][l][csl]
            wvT = np.zeros((E, HC, 65), f32)
            bvrow = np.zeros((1, HC, 65), f32)
            for hh in range(HC):
                wslice = Wv_r[hh * HD:(hh + 1) * HD]          # [64, E]
                wvT[:, hh, :64] = (wslice * g1[None, :] * s).T
                bvrow[0, hh, :64] = wslice @ b1v * s
                bvrow[0, hh, 64] = 1.0
            bq = (Wq_r @ b1v * s).reshape(2, 128)
            bk = (Wk_r @ b1v * s).reshape(2, 128)
            bqk = np.stack([bq[0], bq[1], bk[0], bk[1]], axis=1)   # [128, 4]
            bfv = (g["W1"][l][fsl] @ b2v + g["b1"][l][fsl]).reshape(ET, 128)
            b2q = (g["b2"][l] / TPD).reshape(ET, 128)
            bf2 = np.concatenate([bfv.T, b2q.T], axis=1)           # [128, 16]
            d = {
                "wqA": merge_et((Wq_r * g1[None, :] * s).T, C),
                "wkA": merge_et((Wk_r * g1[None, :] * s).T, C),
                "wvA": merge_et(wvT.reshape(E, HC * 65), VW),
                "bqk": bqk,
                "bvrow": bvrow.reshape(1, VW),
                "wpA": np.ascontiguousarray(
                    (g["Wp"][l][:, csl] * s).T.reshape(2, 128, E)
                    .transpose(1, 0, 2).reshape(128, 2 * E)),
                "w1A": merge_et((g["W1"][l][fsl] * g2[None, :]).T, F),
                "w2A": np.ascontiguousarray(
                    g["W2"][l][:, fsl].T.reshape(ET, 128, E)
                    .transpose(1, 0, 2).reshape(128, ET * E)),
                "bf2": bf2,
            }
            lay.append(d)
        per_layer.append(lay)

    embA, hbias = [], []
    for r in range(TPD):
        vsl = slice(VC * r, VC * (r + 1))
        e = (g["tok_emb"][vsl] * g["lnf_g"][None, :]).T       # [E, 8000]
        ep = np.zeros((E, VCP), f32)
        ep[:, :VC] = e
        # [E, VCP] -> [VCP, E] tiled: embA[vt*128+p, et*128+vv] = ep[et*128+p, vt*128+vv]
        ea = ep.reshape(ET, 128, VCP // 128, 128).transpose(2, 1, 0, 3).reshape(VCP, E)
        embA.append(np.ascontiguousarray(ea))
        hbias.append(g["tok_emb"][vsl] @ g["lnf_b"] + g["head_b"][vsl])

    x0 = g["tok_emb"][idx] + g["pos_emb"][None, :T]           # [2, T, E]

    in_maps = []
    for c in range(NCORES):
        gb, r = c // TPD, c % TPD
        x0T = x0[gb].T                                         # [E, T]
        x0A = (x0T.reshape(ET, 128, NCH, CH).transpose(2, 1, 0, 3)
               .reshape(NCH, 128, ET * CH))
        m = {
            "x0A": np.ascontiguousarray(x0A).astype(bf16),
            "embA": embA[r].astype(bf16),
            "masks": mask_j.astype(bf16),
            "invE": np.full((128, 1), 1.0 / E, bf16),
            "ones128": np.ones((1, 128), bf16),
            "ones64": np.ones((1, 64), bf16),
        }
        for l in range(L):
            d = per_layer[l][r]
            for k, v_ in d.items():
                m[f"{k}{l}"] = v_.astype(f32) if k in ("bqk", "bf2") else v_.astype(bf16)
        in_maps.append(m)
    return in_maps, hbias


LAST_RESULT = None


def kernel(**inputs):
    global _NC, LAST_RESULT
    if _NC is None:
        _NC = _build_program()
    in_maps, hbias = _host_prep(inputs)
    import os
    trace = bool(os.environ.get("KBENCH_TRACE"))
    kw = {}
    if trace:
        import tempfile
        td = os.environ.get("KBENCH_TRACE_DIR")
        if td:
            os.makedirs(td, exist_ok=True)
        else:
            td = tempfile.mkdtemp(prefix="kbench_trace_")
        kw = dict(trace=True, tmpdir=td)
    res = run_bass_kernel_spmd(_NC, in_maps, list(range(NCORES)), **kw)
    LAST_RESULT = res
    B = 2
    logits = np.empty((B, T, V), np.float32)
    for c in range(NCORES):
        gb, r = c // TPD, c % TPD
        lt = np.asarray(res.results[c]["logitsT"][:VC, :], np.float32)  # [8000, T]
        logits[gb, :, VC * r:VC * (r + 1)] = lt.T + hbias[r][None, :]
    return logits


# revision 23
# speedup vs baseline: 1.0189x; 1.0189x over previous
"""Trainium2 Bass kernel for nn_CLM_23038204575917 (dense transformer CLM).

Sharding: DP=2 over batch x TP=4 within batch group.
  core c (0..7): batch g = c//4, TP rank r = c%4.
  - attention heads: 4 per core (of 16), head-dim 64 -> 256 attn channels
  - FFN hidden: 1024 per core (of 4096)
  - lm_head vocab: 8000 per core (of 32000), padded to 8064
Activations kept transposed [E, tok] in bf16; LN gamma/beta folded into
weights host-side; softmax without max-subtraction (scores tiny), causal
mask applied multiplicatively after exp; softmax denom via ones-column
in V; normalization via DVE divide (2x mode) instead of the slow serial
InstReciprocal; all row-broadcasts ride bf16 ones-matmuls (fp32 matmuls
run LOW/HIGH double passes - avoided).

v2 layout/scheduling:
  - layer emitted as 4 chunk-passes (attn c0, attn c1, ffn c0, ffn c1),
    each ending in its AllReduce, so every AR overlaps the next pass's
    AR-independent PE work (the static per-engine instruction order
    stalls head-of-line otherwise).
  - x/h/dsb/ds are merged [128, 8*CH] tiles; AR bounce is ONE DMA each
    way (was 8), readback on the scalar queue, bounce-in on gpsimd.
  - weights DMA'd as single merged [128, 8*X] tiles per matrix.
  - lm_head: emb pre-packed host-side to [8064, 1024] so each vocab tile
    is one 256KB DMA; ci-major loop so chunk-0 logits hide the last AR;
    logits written bf16 (upcast host-side).
"""

import contextlib
import ctypes
import sys
import types

import numpy as np

sys.path.insert(0, "/opt/trn_rl_repo")

import ml_dtypes

bf16 = ml_dtypes.bfloat16

# ---------------------------------------------------------------- ntff hook
# Allows run_bass_kernel_spmd(trace=True) / BASS_TRACE=1 to profile through
# the axon PJRT plugin even though the image's antenv lacks axon_hooks.
if "antenv.axon_hooks" not in sys.modules:
    def _ntff_profile_via_ctypes(so_path):
        try:
            lib = ctypes.CDLL(so_path)
        except OSError:
            return None
        if not hasattr(lib, "axon_start_nrt_profile"):
            return None
        lib.axon_start_nrt_profile.argtypes = [ctypes.POINTER(ctypes.c_int64), ctypes.c_size_t]
        lib.axon_start_nrt_profile.restype = ctypes.c_int64
        lib.axon_stop_nrt_profile.argtypes = [ctypes.c_char_p]
        lib.axon_stop_nrt_profile.restype = ctypes.c_int64

        @contextlib.contextmanager
        def _hook(output_dir, device_ids):
            import jax
            jax.devices()
            if device_ids:
                ids = (ctypes.c_int64 * len(device_ids))(*device_ids)
                rc = lib.axon_start_nrt_profile(ids, len(device_ids))
            else:
                rc = lib.axon_start_nrt_profile(None, 0)
            if rc != 0:
                raise RuntimeError(f"axon_start_nrt_profile rc={rc}")
            try:
                yield
            finally:
                n = lib.axon_stop_nrt_profile(str(output_dir).encode())
                print(f"ntff profile: {n} file(s) -> {output_dir}", file=sys.stderr)

        return _hook

    _mod = types.ModuleType("antenv.axon_hooks")
    _mod._hook = _ntff_profile_via_ctypes("/opt/axon/libaxon_pjrt.so")
    _mod.get_axon_ntff_profile_hook = lambda: _mod._hook
    _mod.set_axon_ntff_profile_hook = lambda h: setattr(_mod, "_hook", h)
    sys.modules["antenv.axon_hooks"] = _mod

import concourse.bass as bass
import concourse.tile as tile
from concourse import mybir
from concourse.bass_utils import run_bass_kernel_spmd

DT = mybir.dt
AF = mybir.ActivationFunctionType
ALU = mybir.AluOpType

# Model dims
V, T, E, H, L, FFD = 32000, 1024, 1024, 16, 4, 4096
HD = 64
NCORES = 8
TPD = 4                  # tensor-parallel degree within a batch group
HC = H // TPD            # heads per core = 4
C = HC * HD              # attn channels per core = 256
F = FFD // TPD           # ffn hidden per core = 1024
VC = V // TPD            # vocab slice per core = 8000
VCP = 8064               # padded to 63*128
ET = E // 128            # 8 e-tiles
NCH = 2                  # token chunks of 512
CH = 512
VW = 260                 # HC * 65 v columns (64 dims + ones col per head)
GROUPS = [[0, 1, 2, 3], [4, 5, 6, 7]]


def _split_sync_waits(nc, max_waits=1):
    """This env's walrus rejects >1 sem-wait per instruction; move excess
    waits onto same-engine NoOps inserted just before."""
    for fn in nc.m.functions:
        for bb in fn.blocks:
            new_list = []
            for ins in bb.instructions:
                si = ins.sync_info
                if si is not None and si.on_wait and len(si.on_wait) > max_waits:
                    waits = list(si.on_wait)
                    extra, keep = waits[:-max_waits], waits[-max_waits:]
                    for k in range(0, len(extra), max_waits):
                        nop = mybir.InstNoOp(name=f"{ins.name}-ws{k}", ins=[], outs=[])
                        nop.engine = ins.engine
                        nop.sync_info = mybir.SyncInfo(
                            on_wait=extra[k:k + max_waits], on_update=[])
                        new_list.append(nop)
                    si.on_wait = keep
                new_list.append(ins)
            bb.instructions[:] = new_list


def _build_program():
    nc = bass.Bass()
    inp = {}

    def din(name, shape, dt=DT.bfloat16):
        inp[name] = nc.dram_tensor(name, list(shape), dt, kind="ExternalInput")
        return inp[name]

    x0A_d = din("x0A", (NCH, 128, ET * CH))
    embA_d = din("embA", (VCP, ET * 128))
    masks_d = din("masks", (4, 128, CH))
    invE_d = din("invE", (128, 1))
    ones128_d = din("ones128", (1, 128))
    ones64_d = din("ones64", (1, 64))
    for l in range(L):
        din(f"wqA{l}", (128, ET * C)); din(f"wkA{l}", (128, ET * C))
        din(f"wvA{l}", (128, ET * VW))
        din(f"bqk{l}", (128, 4), DT.float32)       # cols: bq0,bq1,bk0,bk1
        din(f"bvrow{l}", (1, VW))
        din(f"wpA{l}", (128, 2 * E))
        din(f"w1A{l}", (128, ET * F))
        din(f"w2A{l}", (128, ET * E))
        din(f"bf2{l}", (128, 2 * ET), DT.float32)  # cols 0..7 bf, 8..15 b2q
    out_d = nc.dram_tensor("logitsT", [VCP, T], DT.bfloat16, kind="ExternalOutput")

    with tile.TileContext(nc) as tc, contextlib.ExitStack() as ctx:
        cpool = ctx.enter_context(tc.tile_pool(name="const", bufs=1))
        xpool = ctx.enter_context(tc.tile_pool(name="x", bufs=1))
        hpool = ctx.enter_context(tc.tile_pool(name="h", bufs=1))
        wpool = ctx.enter_context(tc.tile_pool(name="w", bufs=1))
        wbig = ctx.enter_context(tc.tile_pool(name="wbig", bufs=1))
        qkv = ctx.enter_context(tc.tile_pool(name="qkv", bufs=1))
        wexpp = ctx.enter_context(tc.tile_pool(name="wexp", bufs=1))
        opool = ctx.enter_context(tc.tile_pool(name="o", bufs=1))
        dpool = ctx.enter_context(tc.tile_pool(name="d", bufs=1))
        rowp = ctx.enter_context(tc.tile_pool(name="rows", bufs=1))
        lsbp = ctx.enter_context(tc.tile_pool(name="lsb", bufs=3))
        embp = ctx.enter_context(tc.tile_pool(name="emb", bufs=3))
        dram = ctx.enter_context(tc.tile_pool(name="dram", bufs=1, space="DRAM"))
        ps_mm = ctx.enter_context(tc.tile_pool(name="psmm", bufs=2, space="PSUM"))
        ps_w = ctx.enter_context(tc.tile_pool(name="psw", bufs=3, space="PSUM"))
        ps_o = ctx.enter_context(tc.tile_pool(name="pso", bufs=2, space="PSUM"))
        ps_s = ctx.enter_context(tc.tile_pool(name="pss", bufs=1, space="PSUM"))

        # ---- constants
        invE = cpool.tile([128, 1], DT.bfloat16, tag="invE", name="invE")
        nc.sync.dma_start(invE[:], invE_d[:])
        ones128 = cpool.tile([1, 128], DT.bfloat16, tag="ones128", name="ones128")
        nc.sync.dma_start(ones128[:], ones128_d[:])
        ones64 = cpool.tile([1, 64], DT.bfloat16, tag="ones64", name="ones64")
        nc.sync.dma_start(ones64[:], ones64_d[:])
        eps128 = cpool.tile([128, 1], DT.float32, tag="eps128", name="eps128")
        nc.gpsimd.memset(eps128[:], 1e-5)
        masks = [cpool.tile([128, CH], DT.bfloat16, tag=f"mask{j}", name=f"mask{j}") for j in range(4)]
        for j in range(4):
            nc.sync.dma_start(masks[j][:], masks_d[j])

        # ---- residual, merged per-chunk tiles x[ci] = [128, 8*CH]
        x = [xpool.tile([128, ET * CH], DT.bfloat16, tag=f"x{ci}", name=f"x{ci}")
             for ci in range(NCH)]
        for ci in range(NCH):
            nc.sync.dma_start(x[ci][:], x0A_d[ci])

        def esl(et):
            return slice(et * CH, (et + 1) * CH)

        def ln_chunk(hdst, ci):
            """hdst[:, et*CH:(et+1)*CH] = (x - mu) / sd for token chunk ci.

            Tile sums over the 8 e-tiles ride DVE tree-adds (PE does just 2
            reduction matmuls instead of 16)."""
            xc = x[ci]
            mom = ps_s.tile([33, CH], DT.float32, tag="mom", name="mom")
            mu_ps, m2_ps = mom[0:1, :], mom[32:33, :]
            for et in range(ET):
                nc.tensor.matmul(mu_ps, invE[:], xc[:, esl(et)],
                                 start=(et == 0), stop=(et == ET - 1))
            for et in range(ET):
                xsq = hpool.tile([128, CH], DT.bfloat16, tag="xsq", bufs=3, name="xsq")
                nc.vector.tensor_tensor(xsq[:], xc[:, esl(et)], xc[:, esl(et)], op=ALU.mult)
                nc.tensor.matmul(m2_ps, invE[:], xsq[:],
                                 start=(et == 0), stop=(et == ET - 1))
            mu = rowp.tile([1, CH], DT.float32, tag="mu_sb", name="mu_sb")
            nc.scalar.activation(mu[:], mu_ps, AF.Identity)
            mu2 = rowp.tile([1, CH], DT.float32, tag="mu2_sb", name="mu2_sb")
            nc.vector.tensor_tensor(mu2[:], mu[:], mu[:], op=ALU.mult)
            var = rowp.tile([1, CH], DT.float32, tag="var_sb", name="var_sb")
            nc.vector.tensor_tensor(var[:], m2_ps, mu2[:], op=ALU.subtract)
            sd = rowp.tile([1, CH], DT.float32, tag="sd_sb", name="sd_sb")
            nc.scalar.activation(sd[:], var[:], AF.Sqrt, bias=eps128[0:1, :])
            a32 = rowp.tile([1, CH], DT.float32, tag="a32", name="a32")
            nc.vector.reciprocal(a32[:], sd[:])
            a16 = rowp.tile([1, CH], DT.bfloat16, tag="a16", name="a16")
            nc.scalar.activation(a16[:], a32[:], AF.Identity)
            c16 = rowp.tile([1, CH], DT.bfloat16, tag="c16", name="c16")
            nc.vector.tensor_tensor(c16[:], mu[:], a32[:], op=ALU.mult)
            ab_ps = ps_w.tile([128, CH], DT.float32, tag="w", name="ab")
            nc.tensor.matmul(ab_ps[:], ones128[:], a16[:], start=True, stop=True)
            cb_ps = ps_w.tile([128, CH], DT.float32, tag="w", name="cb")
            nc.tensor.matmul(cb_ps[:], ones128[:], c16[:], start=True, stop=True)
            a_b = rowp.tile([128, CH], DT.bfloat16, tag="ab_sb", bufs=2, name="ab_sb")
            nc.scalar.activation(a_b[:], ab_ps[:], AF.Identity)
            c_b = rowp.tile([128, CH], DT.bfloat16, tag="cb_sb", bufs=2, name="cb_sb")
            nc.scalar.activation(c_b[:], cb_ps[:], AF.Identity)
            for et in range(ET):
                tmp = hpool.tile([128, CH], DT.bfloat16, tag="lnt", name="lnt")
                nc.vector.tensor_tensor(tmp[:], xc[:, esl(et)], a_b[:], op=ALU.mult)
                nc.vector.tensor_tensor(hdst[:, esl(et)], tmp[:], c_b[:], op=ALU.subtract)

        def ar_block(dsb, l, phase, ci):
            """bounce dsb -> AllReduce(group of 4) -> residual add into x[ci]."""
            dloc = dram.tile([128, ET * CH], DT.bfloat16,
                             tag=f"dloc_{phase}{l}_{ci}", name="dloc")
            dred = dram.tile([128, ET * CH], DT.bfloat16,
                             tag=f"dred_{phase}{l}_{ci}", name="dred")
            nc.gpsimd.dma_start(dloc[:], dsb[:])
            nc.gpsimd.collective_compute(
                "AllReduce", ALU.add, replica_groups=GROUPS,
                ins=[dloc.opt()], outs=[dred.opt()])
            ds = dpool.tile([128, ET * CH], DT.bfloat16, tag="dsA", bufs=2, name="dsA")
            nc.scalar.dma_start(ds[:], dred[:])
            for p in range(4):
                psl2 = slice(2 * p * CH, (2 * p + 2) * CH)
                nc.vector.tensor_tensor(x[ci][:, psl2], x[ci][:, psl2],
                                        ds[:, psl2], op=ALU.add)

        # persistent per-layer qkv tiles
        for l in range(L):
            # ---- layer weights to SBUF (merged single DMAs)
            wq = wpool.tile([128, ET * C], DT.bfloat16, tag="wq", name="wq")
            wk = wpool.tile([128, ET * C], DT.bfloat16, tag="wk", name="wk")
            wv = wpool.tile([128, ET * VW], DT.bfloat16, tag="wv", name="wv")
            wp = wpool.tile([128, 2 * E], DT.bfloat16, tag="wp", name="wp")
            bqk = wpool.tile([128, 4], DT.float32, tag="bqk", name="bqk")
            bvrow = wpool.tile([1, VW], DT.bfloat16, tag="bvrow", name="bvrow")
            nc.sync.dma_start(wq[:], inp[f"wqA{l}"][:])
            nc.sync.dma_start(wk[:], inp[f"wkA{l}"][:])
            nc.sync.dma_start(wv[:], inp[f"wvA{l}"][:])
            nc.sync.dma_start(wp[:], inp[f"wpA{l}"][:])
            nc.sync.dma_start(bqk[:], inp[f"bqk{l}"][:])
            nc.sync.dma_start(bvrow[:], inp[f"bvrow{l}"][:])
            w1 = wbig.tile([128, ET * F], DT.bfloat16, tag="w1", name="w1")
            w2 = wbig.tile([128, ET * E], DT.bfloat16, tag="w2", name="w2")
            bf2 = wpool.tile([128, 2 * ET], DT.float32, tag="bf2", name="bf2")
            nc.sync.dma_start(w1[:], inp[f"w1A{l}"][:])
            nc.sync.dma_start(w2[:], inp[f"w2A{l}"][:])
            nc.sync.dma_start(bf2[:], inp[f"bf2{l}"][:])

            qT = [[qkv.tile([128, CH], DT.bfloat16, tag=f"qT{ct}_{ci}", name=f"qT{ct}_{ci}")
                   for ci in range(NCH)] for ct in range(2)]
            kT = [[qkv.tile([128, CH], DT.bfloat16, tag=f"kT{ct}_{ci}", name=f"kT{ct}_{ci}")
                   for ci in range(NCH)] for ct in range(2)]
            v = [qkv.tile([128, VW], DT.bfloat16, tag=f"v{tt}", name=f"v{tt}")
                 for tt in range(8)]

            # ======== attention passes, one chunk at a time ========
            for ci in range(NCH):
                h = hpool.tile([128, ET * CH], DT.bfloat16, tag=f"h{ci}", name=f"h{ci}")
                ln_chunk(h, ci)

                # Q, K projections for this chunk
                for ct in range(2):
                    pq = ps_mm.tile([128, CH], DT.float32, tag="mm", name="mm")
                    for et in range(ET):
                        nc.tensor.matmul(pq[:], wq[:, et * C + ct * 128: et * C + (ct + 1) * 128],
                                         h[:, esl(et)], start=(et == 0), stop=(et == ET - 1))
                    nc.scalar.activation(qT[ct][ci][:], pq[:], AF.Identity,
                                         bias=bqk[:, ct:ct + 1])
                    pk = ps_mm.tile([128, CH], DT.float32, tag="mm", name="mm")
                    for et in range(ET):
                        nc.tensor.matmul(pk[:], wk[:, et * C + ct * 128: et * C + (ct + 1) * 128],
                                         h[:, esl(et)], start=(et == 0), stop=(et == ET - 1))
                    nc.scalar.activation(kT[ct][ci][:], pk[:], AF.Identity,
                                         bias=bqk[:, 2 + ct:3 + ct])

                # V (token-major, with ones column) for this chunk's 4 tiles
                for tt in range(4 * ci, 4 * ci + 4):
                    lsl = slice((tt % 4) * 128, (tt % 4) * 128 + 128)
                    pv = ps_mm.tile([128, VW], DT.float32, tag="mm", name="mm")
                    for et in range(ET):
                        nc.tensor.matmul(pv[:], h[:, et * CH + (tt % 4) * 128: et * CH + (tt % 4) * 128 + 128],
                                         wv[:, et * VW:(et + 1) * VW],
                                         start=(et == 0), stop=False)
                    nc.tensor.matmul(pv[:], ones128[:], bvrow[:], start=False, stop=True)
                    nc.vector.tensor_copy(v[tt][:], pv[:])

                # attention for this chunk; head-pairs packed on PE row groups
                o2 = [opool.tile([128, CH], DT.bfloat16, tag=f"o2_{hp}_{ci}", name=f"o2_{hp}_{ci}")
                      for hp in range(2)]
                nkt = 4 * ci + 4
                for hp in range(2):
                    ct = hp
                    opsA = ps_o.tile([65, CH], DT.float32, tag="o", name="oA")
                    opsB = ps_o.tile([65, CH], DT.float32, tag="o", name="oB")
                    for kt in range(nkt):
                        j = kt - 4 * ci
                        wexs = []
                        for sub in range(2):
                            psl = slice(64 * sub, 64 * sub + 64)
                            kcj, klo = kt // 4, (kt % 4) * 128
                            pw = ps_w.tile([128, CH], DT.float32, tag="w", name="w")
                            nc.tensor.matmul(pw[:], kT[ct][kcj][psl, klo:klo + 128],
                                             qT[ct][ci][psl, :], start=True, stop=True)
                            wex = wexpp.tile([128, CH], DT.bfloat16, tag="wx", bufs=6,
                                             name=f"wx{kt}_{sub}")
                            if j >= 0:
                                tmp = wexpp.tile([128, CH], DT.bfloat16, tag="wxt",
                                                 bufs=2, name="wxt")
                                nc.scalar.activation(tmp[:], pw[:], AF.Exp, scale=0.125)
                                nc.vector.tensor_tensor(wex[:], tmp[:], masks[j][:],
                                                        op=ALU.mult)
                            else:
                                nc.scalar.activation(wex[:], pw[:], AF.Exp, scale=0.125)
                            wexs.append(wex)
                        hA, hB = 2 * hp, 2 * hp + 1
                        nc.tensor.matmul(opsA[:], v[kt][:, hA * 65:(hA + 1) * 65], wexs[0][:],
                                         start=(kt == 0), stop=(kt == nkt - 1))
                        nc.tensor.matmul(opsB[:], v[kt][:, hB * 65:(hB + 1) * 65], wexs[1][:],
                                         start=(kt == 0), stop=(kt == nkt - 1))
                    for sub, ops in ((0, opsA), (1, opsB)):
                        psl = slice(64 * sub, 64 * sub + 64)
                        s32 = rowp.tile([1, CH], DT.float32, tag="s32", bufs=2, name="s32")
                        nc.scalar.activation(s32[:], ops[64:65, :], AF.Identity)
                        r32 = rowp.tile([1, CH], DT.float32, tag="r32", bufs=2, name="r32")
                        nc.vector.reciprocal(r32[:], s32[:])
                        r16 = rowp.tile([1, CH], DT.bfloat16, tag="r16", bufs=2, name="r16")
                        nc.scalar.activation(r16[:], r32[:], AF.Identity)
                        rb = ps_w.tile([64, CH], DT.float32, tag="w", name="rb")
                        nc.tensor.matmul(rb[:], ones64[:], r16[:], start=True, stop=True)
                        orw = rowp.tile([64, CH], DT.bfloat16, tag="orw", bufs=2, name="orw")
                        nc.scalar.activation(orw[:], ops[0:64, :], AF.Identity)
                        nc.vector.tensor_tensor(o2[hp][psl, :], orw[:], rb[:], op=ALU.mult)

                # proj for this chunk -> dsb -> AR -> residual
                dsb = dpool.tile([128, ET * CH], DT.bfloat16, tag="dsbA", bufs=2,
                                 name="dsbA")
                for et in range(ET):
                    pslE = slice(et * 128, (et + 1) * 128)
                    pd = ps_mm.tile([128, CH], DT.float32, tag="mm", name="mm")
                    nc.tensor.matmul(pd[:], wp[:, 0 * E + et * 128: 0 * E + (et + 1) * 128],
                                     o2[0][:], start=True, stop=False)
                    nc.tensor.matmul(pd[:], wp[:, 1 * E + et * 128: 1 * E + (et + 1) * 128],
                                     o2[1][:], start=False, stop=True)
                    nc.scalar.activation(dsb[:, esl(et)], pd[:], AF.Identity)
                ar_block(dsb, l, "a", ci)

            # ======== FFN passes, one chunk at a time ========
            for ci in range(NCH):
                h2 = hpool.tile([128, ET * CH], DT.bfloat16, tag=f"h{ci}", name=f"h2_{ci}")
                ln_chunk(h2, ci)
                ff = [dpool.tile([128, CH], DT.bfloat16, tag=f"ff{ft}", bufs=2,
                                 name=f"ff{ft}") for ft in range(ET)]
                for ft in range(ET):
                    pf = ps_mm.tile([128, CH], DT.float32, tag="mm", name="mm")
                    for et in range(ET):
                        nc.tensor.matmul(pf[:], w1[:, et * F + ft * 128: et * F + (ft + 1) * 128],
                                         h2[:, esl(et)], start=(et == 0), stop=(et == ET - 1))
                    nc.scalar.activation(ff[ft][:], pf[:], AF.Relu, bias=bf2[:, ft:ft + 1])
                dsb2 = dpool.tile([128, ET * CH], DT.bfloat16, tag="dsbA", bufs=2,
                                  name="dsb2")
                for et in range(ET):
                    pd = ps_mm.tile([128, CH], DT.float32, tag="mm", name="mm")
                    for ft in range(ET):
                        nc.tensor.matmul(pd[:], w2[:, ft * E + et * 128: ft * E + (et + 1) * 128],
                                         ff[ft][:], start=(ft == 0), stop=(ft == ET - 1))
                    nc.scalar.activation(dsb2[:, esl(et)], pd[:], AF.Identity,
                                         bias=bf2[:, ET + et:ET + et + 1])
                ar_block(dsb2, l, "f", ci)

        # ---- final LN + lm_head (ci-major so chunk 0 hides the last AR)
        for ci in range(NCH):
            hf = hpool.tile([128, ET * CH], DT.bfloat16, tag=f"h{ci}", name=f"hf{ci}")
            ln_chunk(hf, ci)
            sl = slice(ci * CH, (ci + 1) * CH)
            for vt in range(VCP // 128):
                vsl = slice(vt * 128, (vt + 1) * 128)
                emb = embp.tile([128, ET * 128], DT.bfloat16, tag="emb", name="emb")
                if vt % 2 == 0:
                    nc.sync.dma_start(emb[:], embA_d[vsl, :])
                else:
                    nc.scalar.dma_start(emb[:], embA_d[vsl, :])
                pool = ps_mm if vt % 2 == 0 else ps_w
                pl = pool.tile([128, CH], DT.float32,
                               tag=("mm" if vt % 2 == 0 else "w"), name="pl")
                for et in range(ET):
                    nc.tensor.matmul(pl[:], emb[:, et * 128:(et + 1) * 128],
                                     hf[:, esl(et)], start=(et == 0), stop=(et == ET - 1))
                lsb = lsbp.tile([128, CH], DT.bfloat16, tag="lsb", name="lsb")
                if vt % 2 == 0:
                    nc.scalar.activation(lsb[:], pl[:], AF.Identity)
                else:
                    nc.vector.tensor_copy(lsb[:], pl[:])
                nc.gpsimd.dma_start(out_d[vsl, sl], lsb[:])

    _split_sync_waits(nc)
    return nc


_NC = None


def _host_prep(inputs):
    """Fold LN params into weights, build per-core input maps."""
    f32 = np.float32

    def as_f32(vv):
        a = np.asarray(vv)
        return a if a.dtype in (np.int64, np.int32) else np.asarray(a, f32)

    g = {k: as_f32(vv) for k, vv in inputs.items()}
    idx = np.asarray(inputs["idx"])
    s = f32(E) ** -0.5

    mask_j = np.zeros((4, 128, CH), f32)
    q_idx = np.arange(CH)[None, :]
    k_idx = np.arange(128)[:, None]
    for j in range(4):
        mask_j[j] = (q_idx >= 128 * j + k_idx).astype(f32)

    def merge_et(mat, ncols):
        # [E, ncols] -> [128, ET*ncols] with et blocks along free axis
        return np.ascontiguousarray(
            mat.reshape(ET, 128, ncols).transpose(1, 0, 2).reshape(128, ET * ncols))

    per_layer = []
    for l in range(L):
        g1, b1v = g["ln1_g"][l], g["ln1_b"][l]
        g2, b2v = g["ln2_g"][l], g["ln2_b"][l]
        lay = []
        for r in range(TPD):
            csl = slice(C * r, C * (r + 1))
            fsl = slice(F * r, F * (r + 1))
            Wq_r, Wk_r, Wv_r = g["Wq"][l][csl], g["Wk"][l][csl], g["Wv"][l][csl]
            wvT = np.zeros((E, HC, 65), f32)
            bvrow = np.zeros((1, HC, 65), f32)
            for hh in range(HC):
                wslice = Wv_r[hh * HD:(hh + 1) * HD]          # [64, E]
                wvT[:, hh, :64] = (wslice * g1[None, :] * s).T
                bvrow[0, hh, :64] = wslice @ b1v * s
                bvrow[0, hh, 64] = 1.0
            bq = (Wq_r @ b1v * s).reshape(2, 128)
            bk = (Wk_r @ b1v * s).reshape(2, 128)
            bqk = np.stack([bq[0], bq[1], bk[0], bk[1]], axis=1)   # [128, 4]
            bfv = (g["W1"][l][fsl] @ b2v + g["b1"][l][fsl]).reshape(ET, 128)
            b2q = (g["b2"][l] / TPD).reshape(ET, 128)
            bf2 = np.concatenate([bfv.T, b2q.T], axis=1)           # [128, 16]
            d = {
                "wqA": merge_et((Wq_r * g1[None, :] * s).T, C),
                "wkA": merge_et((Wk_r * g1[None, :] * s).T, C),
                "wvA": merge_et(wvT.reshape(E, HC * 65), VW),
                "bqk": bqk,
                "bvrow": bvrow.reshape(1, VW),
                "wpA": np.ascontiguousarray(
                    (g["Wp"][l][:, csl] * s).T.reshape(2, 128, E)
                    .transpose(1, 0, 2).reshape(128, 2 * E)),
                "w1A": merge_et((g["W1"][l][fsl] * g2[None, :]).T, F),
                "w2A": np.ascontiguousarray(
                    g["W2"][l][:, fsl].T.reshape(ET, 128, E)
                    .transpose(1, 0, 2).reshape(128, ET * E)),
                "bf2": bf2,
            }
            lay.append(d)
        per_layer.append(lay)

    embA, hbias = [], []
    for r in range(TPD):
        vsl = slice(VC * r, VC * (r + 1))
        e = (g["tok_emb"][vsl] * g["lnf_g"][None, :]).T       # [E, 8000]
        ep = np.zeros((E, VCP), f32)
        ep[:, :VC] = e
        # [E, VCP] -> [VCP, E] tiled: embA[vt*128+p, et*128+vv] = ep[et*128+p, vt*128+vv]
        ea = ep.reshape(ET, 128, VCP // 128, 128).transpose(2, 1, 0, 3).reshape(VCP, E)
        embA.append(np.ascontiguousarray(ea))
        hbias.append(g["tok_emb"][vsl] @ g["lnf_b"] + g["head_b"][vsl])

    x0 = g["tok_emb"][idx] + g["pos_emb"][None, :T]           # [2, T, E]

    in_maps = []
    for c in range(NCORES):
        gb, r = c // TPD, c % TPD
        x0T = x0[gb].T                                         # [E, T]
        x0A = (x0T.reshape(ET, 128, NCH, CH).transpose(2, 1, 0, 3)
               .reshape(NCH, 128, ET * CH))
        m = {
            "x0A": np.ascontiguousarray(x0A).astype(bf16),
            "embA": embA[r].astype(bf16),
            "masks": mask_j.astype(bf16),
            "invE": np.full((128, 1), 1.0 / E, bf16),
            "ones128": np.ones((1, 128), bf16),
            "ones64": np.ones((1, 64), bf16),
        }
        for l in range(L):
            d = per_layer[l][r]
            for k, v_ in d.items():
                m[f"{k}{l}"] = v_.astype(f32) if k in ("bqk", "bf2") else v_.astype(bf16)
        in_maps.append(m)
    return in_maps, hbias


LAST_RESULT = None


def kernel(**inputs):
    global _NC, LAST_RESULT
    if _NC is None:
        _NC = _build_program()
    in_maps, hbias = _host_prep(inputs)
    import os
    trace = bool(os.environ.get("KBENCH_TRACE"))
    kw = {}
    if trace:
        import tempfile
        td = os.environ.get("KBENCH_TRACE_DIR")
        if td:
            os.makedirs(td, exist_ok=True)
        else:
            td = tempfile.mkdtemp(prefix="kbench_trace_")
        kw = dict(trace=True, tmpdir=td)
    res = run_bass_kernel_spmd(_NC, in_maps, list(range(NCORES)), **kw)
    LAST_RESULT = res
    B = 2
    logits = np.empty((B, T, V), np.float32)
    for c in range(NCORES):
        gb, r = c // TPD, c % TPD
        lt = np.asarray(res.results[c]["logitsT"][:VC, :], np.float32)  # [8000, T]
        logits[gb, :, VC * r:VC * (r + 1)] = lt.T + hbias[r][None, :]
    return logits


# revision 28
# speedup vs baseline: 1.0537x; 1.0341x over previous
"""Trainium2 Bass kernel for nn_CLM_23038204575917 (dense transformer CLM).

Sharding: DP=2 over batch x TP=4 within batch group.
  core c (0..7): batch g = c//4, TP rank r = c%4.
  - attention heads: 4 per core (of 16), head-dim 64 -> 256 attn channels
  - FFN hidden: 1024 per core (of 4096)
  - lm_head vocab: 8000 per core (of 32000), padded to 8064
Activations kept transposed [E, tok] in bf16; LN gamma/beta folded into
weights host-side; softmax without max-subtraction (scores tiny), causal
mask applied multiplicatively after exp; softmax denom via ones-column
in V; all row-broadcasts ride bf16 ones-matmuls (fp32 matmuls run
LOW/HIGH double passes on this HW - avoided).

v2 layout/scheduling:
  - layer emitted as 4 chunk-passes (attn c0, attn c1, ffn c0, ffn c1),
    each ending in its AllReduce, so every AR overlaps the next pass's
    AR-independent PE work (the static per-engine instruction order
    stalls head-of-line otherwise).
  - x/h/dsb/ds are merged [128, 8*CH] tiles; AR bounce is ONE DMA each
    way (was 8), readback on the scalar queue, bounce-in on gpsimd.
  - weights DMA'd as single merged [128, 8*X] tiles per matrix.
  - lm_head: emb pre-packed host-side to [8064, 1024] so each vocab tile
    is one 256KB DMA; ci-major loop so chunk-0 logits hide the last AR;
    logits written bf16 (upcast host-side).
"""

import contextlib
import ctypes
import sys
import types

import numpy as np

sys.path.insert(0, "/opt/trn_rl_repo")

import ml_dtypes

bf16 = ml_dtypes.bfloat16

# ---------------------------------------------------------------- ntff hook
# Allows run_bass_kernel_spmd(trace=True) / BASS_TRACE=1 to profile through
# the axon PJRT plugin even though the image's antenv lacks axon_hooks.
if "antenv.axon_hooks" not in sys.modules:
    def _ntff_profile_via_ctypes(so_path):
        try:
            lib = ctypes.CDLL(so_path)
        except OSError:
            return None
        if not hasattr(lib, "axon_start_nrt_profile"):
            return None
        lib.axon_start_nrt_profile.argtypes = [ctypes.POINTER(ctypes.c_int64), ctypes.c_size_t]
        lib.axon_start_nrt_profile.restype = ctypes.c_int64
        lib.axon_stop_nrt_profile.argtypes = [ctypes.c_char_p]
        lib.axon_stop_nrt_profile.restype = ctypes.c_int64

        @contextlib.contextmanager
        def _hook(output_dir, device_ids):
            import jax
            jax.devices()
            if device_ids:
                ids = (ctypes.c_int64 * len(device_ids))(*device_ids)
                rc = lib.axon_start_nrt_profile(ids, len(device_ids))
            else:
                rc = lib.axon_start_nrt_profile(None, 0)
            if rc != 0:
                raise RuntimeError(f"axon_start_nrt_profile rc={rc}")
            try:
                yield
            finally:
                n = lib.axon_stop_nrt_profile(str(output_dir).encode())
                print(f"ntff profile: {n} file(s) -> {output_dir}", file=sys.stderr)

        return _hook

    _mod = types.ModuleType("antenv.axon_hooks")
    _mod._hook = _ntff_profile_via_ctypes("/opt/axon/libaxon_pjrt.so")
    _mod.get_axon_ntff_profile_hook = lambda: _mod._hook
    _mod.set_axon_ntff_profile_hook = lambda h: setattr(_mod, "_hook", h)
    sys.modules["antenv.axon_hooks"] = _mod

import concourse.bass as bass
import concourse.tile as tile
from concourse import mybir
from concourse.bass_utils import run_bass_kernel_spmd

DT = mybir.dt
AF = mybir.ActivationFunctionType
ALU = mybir.AluOpType

# Model dims
V, T, E, H, L, FFD = 32000, 1024, 1024, 16, 4, 4096
HD = 64
NCORES = 8
TPD = 4                  # tensor-parallel degree within a batch group
HC = H // TPD            # heads per core = 4
C = HC * HD              # attn channels per core = 256
F = FFD // TPD           # ffn hidden per core = 1024
VC = V // TPD            # vocab slice per core = 8000
VCP = 8064               # padded to 63*128
ET = E // 128            # 8 e-tiles
NCH = 2                  # token chunks of 512
CH = 512
VW = 260                 # HC * 65 v columns (64 dims + ones col per head)
GROUPS = [[0, 1, 2, 3], [4, 5, 6, 7]]


def _split_sync_waits(nc, max_waits=1):
    """This env's walrus rejects >1 sem-wait per instruction; move excess
    waits onto same-engine NoOps inserted just before."""
    for fn in nc.m.functions:
        for bb in fn.blocks:
            new_list = []
            for ins in bb.instructions:
                si = ins.sync_info
                if si is not None and si.on_wait and len(si.on_wait) > max_waits:
                    waits = list(si.on_wait)
                    extra, keep = waits[:-max_waits], waits[-max_waits:]
                    for k in range(0, len(extra), max_waits):
                        nop = mybir.InstNoOp(name=f"{ins.name}-ws{k}", ins=[], outs=[])
                        nop.engine = ins.engine
                        nop.sync_info = mybir.SyncInfo(
                            on_wait=extra[k:k + max_waits], on_update=[])
                        new_list.append(nop)
                    si.on_wait = keep
                new_list.append(ins)
            bb.instructions[:] = new_list


def _build_program():
    nc = bass.Bass()
    inp = {}

    def din(name, shape, dt=DT.bfloat16):
        inp[name] = nc.dram_tensor(name, list(shape), dt, kind="ExternalInput")
        return inp[name]

    x0A_d = din("x0A", (NCH, 128, ET * CH))
    embA_d = din("embA", (VCP, ET * 128))
    masks_d = din("masks", (4, 128, CH))
    invE_d = din("invE", (128, 1))
    ones128_d = din("ones128", (1, 128))
    ones64_d = din("ones64", (1, 64))
    for l in range(L):
        din(f"wqA{l}", (128, ET * C)); din(f"wkA{l}", (128, ET * C))
        din(f"wvA{l}", (128, ET * VW))
        din(f"bqk{l}", (128, 4), DT.float32)       # cols: bq0,bq1,bk0,bk1
        din(f"bvrow{l}", (1, VW))
        din(f"wpA{l}", (128, 2 * E))
        din(f"w1A{l}", (128, ET * F))
        din(f"w2A{l}", (128, ET * E))
        din(f"bf2{l}", (128, 2 * ET), DT.float32)  # cols 0..7 bf, 8..15 b2q
    out_d = nc.dram_tensor("logitsT", [VCP, T], DT.bfloat16, kind="ExternalOutput")

    with tile.TileContext(nc) as tc, contextlib.ExitStack() as ctx:
        cpool = ctx.enter_context(tc.tile_pool(name="const", bufs=1))
        xpool = ctx.enter_context(tc.tile_pool(name="x", bufs=1))
        hpool = ctx.enter_context(tc.tile_pool(name="h", bufs=1))
        wpool = ctx.enter_context(tc.tile_pool(name="w", bufs=1))
        wbig = ctx.enter_context(tc.tile_pool(name="wbig", bufs=1))
        qkv = ctx.enter_context(tc.tile_pool(name="qkv", bufs=1))
        wexpp = ctx.enter_context(tc.tile_pool(name="wexp", bufs=1))
        opool = ctx.enter_context(tc.tile_pool(name="o", bufs=1))
        dpool = ctx.enter_context(tc.tile_pool(name="d", bufs=1))
        rowp = ctx.enter_context(tc.tile_pool(name="rows", bufs=1))
        lsbp = ctx.enter_context(tc.tile_pool(name="lsb", bufs=3))
        embp = ctx.enter_context(tc.tile_pool(name="emb", bufs=3))
        dram = ctx.enter_context(tc.tile_pool(name="dram", bufs=1, space="DRAM"))
        ps_mm = ctx.enter_context(tc.tile_pool(name="psmm", bufs=2, space="PSUM"))
        ps_w = ctx.enter_context(tc.tile_pool(name="psw", bufs=3, space="PSUM"))
        ps_o = ctx.enter_context(tc.tile_pool(name="pso", bufs=2, space="PSUM"))
        ps_s = ctx.enter_context(tc.tile_pool(name="pss", bufs=1, space="PSUM"))

        # ---- constants
        invE = cpool.tile([128, 1], DT.bfloat16, tag="invE", name="invE")
        nc.sync.dma_start(invE[:], invE_d[:])
        ones128 = cpool.tile([1, 128], DT.bfloat16, tag="ones128", name="ones128")
        nc.sync.dma_start(ones128[:], ones128_d[:])
        ones64 = cpool.tile([1, 64], DT.bfloat16, tag="ones64", name="ones64")
        nc.sync.dma_start(ones64[:], ones64_d[:])
        eps128 = cpool.tile([128, 1], DT.float32, tag="eps128", name="eps128")
        nc.gpsimd.memset(eps128[:], 1e-5)
        masks = [cpool.tile([128, CH], DT.bfloat16, tag=f"mask{j}", name=f"mask{j}") for j in range(4)]
        for j in range(4):
            nc.sync.dma_start(masks[j][:], masks_d[j])

        # ---- warmup AllReduce: the first collectives of a NEFF run ~15us
        # slower (cold ncfw); burn that cost in the prologue where the CC
        # queue is idle and nothing consumes the result.
        wu_in = dram.tile([128, 64], DT.bfloat16, tag="wu_in", name="wu_in")
        wu_out = dram.tile([128, 64], DT.bfloat16, tag="wu_out", name="wu_out")
        wu_sb = cpool.tile([128, 64], DT.bfloat16, tag="wu_sb", name="wu_sb")
        nc.gpsimd.memset(wu_sb[:], 0.0)
        nc.gpsimd.dma_start(wu_in[:], wu_sb[:])
        nc.gpsimd.collective_compute(
            "AllReduce", ALU.add, replica_groups=GROUPS,
            ins=[wu_in.opt()], outs=[wu_out.opt()])

        # ---- residual, merged per-chunk tiles x[ci] = [128, 8*CH]
        x = [xpool.tile([128, ET * CH], DT.bfloat16, tag=f"x{ci}", name=f"x{ci}")
             for ci in range(NCH)]
        for ci in range(NCH):
            nc.sync.dma_start(x[ci][:], x0A_d[ci])

        def esl(et):
            return slice(et * CH, (et + 1) * CH)

        def ln_chunk(hdst, ci):
            """hdst[:, et*CH:(et+1)*CH] = (x - mu) / sd for token chunk ci.

            Tile sums over the 8 e-tiles ride DVE tree-adds (PE does just 2
            reduction matmuls instead of 16)."""
            xc = x[ci]
            mom = ps_s.tile([33, CH], DT.float32, tag="mom", name="mom")
            mu_ps, m2_ps = mom[0:1, :], mom[32:33, :]
            for et in range(ET):
                nc.tensor.matmul(mu_ps, invE[:], xc[:, esl(et)],
                                 start=(et == 0), stop=(et == ET - 1))
            for et in range(ET):
                xsq = hpool.tile([128, CH], DT.bfloat16, tag="xsq", bufs=3, name="xsq")
                nc.vector.tensor_tensor(xsq[:], xc[:, esl(et)], xc[:, esl(et)], op=ALU.mult)
                nc.tensor.matmul(m2_ps, invE[:], xsq[:],
                                 start=(et == 0), stop=(et == ET - 1))
            # mu broadcast + the 8 subtract TTs run concurrently with the
            # slow reciprocal chain; only the final 8 mult TTs wait on a_b.
            mu = rowp.tile([1, CH], DT.float32, tag="mu_sb", name="mu_sb")
            nc.scalar.activation(mu[:], mu_ps, AF.Identity)
            mu16 = rowp.tile([1, CH], DT.bfloat16, tag="mu16", name="mu16")
            nc.scalar.activation(mu16[:], mu_ps, AF.Identity)
            mub_ps = ps_w.tile([128, CH], DT.float32, tag="w", name="mub")
            nc.tensor.matmul(mub_ps[:], ones128[:], mu16[:], start=True, stop=True)
            mu_b = rowp.tile([128, CH], DT.bfloat16, tag="mub_sb", bufs=2, name="mub_sb")
            nc.scalar.activation(mu_b[:], mub_ps[:], AF.Identity)
            mu2 = rowp.tile([1, CH], DT.float32, tag="mu2_sb", name="mu2_sb")
            nc.vector.tensor_tensor(mu2[:], mu[:], mu[:], op=ALU.mult)
            var = rowp.tile([1, CH], DT.float32, tag="var_sb", name="var_sb")
            nc.vector.tensor_tensor(var[:], m2_ps, mu2[:], op=ALU.subtract)
            sd = rowp.tile([1, CH], DT.float32, tag="sd_sb", name="sd_sb")
            nc.scalar.activation(sd[:], var[:], AF.Sqrt, bias=eps128[0:1, :])
            a32 = rowp.tile([1, CH], DT.float32, tag="a32", name="a32")
            nc.vector.reciprocal(a32[:], sd[:])
            a16 = rowp.tile([1, CH], DT.bfloat16, tag="a16", name="a16")
            nc.scalar.activation(a16[:], a32[:], AF.Identity)
            ab_ps = ps_w.tile([128, CH], DT.float32, tag="w", name="ab")
            nc.tensor.matmul(ab_ps[:], ones128[:], a16[:], start=True, stop=True)
            a_b = rowp.tile([128, CH], DT.bfloat16, tag="ab_sb", bufs=2, name="ab_sb")
            nc.scalar.activation(a_b[:], ab_ps[:], AF.Identity)
            for et in range(ET):
                nc.vector.tensor_tensor(hdst[:, esl(et)], xc[:, esl(et)], mu_b[:],
                                        op=ALU.subtract)
            for et in range(ET):
                nc.vector.tensor_tensor(hdst[:, esl(et)], hdst[:, esl(et)], a_b[:],
                                        op=ALU.mult)

        def ar_block(dsb, l, phase, ci):
            """bounce dsb -> AllReduce(group of 4) -> residual add into x[ci]."""
            dloc = dram.tile([128, ET * CH], DT.bfloat16,
                             tag=f"dloc_{phase}{l}_{ci}", name="dloc")
            dred = dram.tile([128, ET * CH], DT.bfloat16,
                             tag=f"dred_{phase}{l}_{ci}", name="dred")
            nc.gpsimd.dma_start(dloc[:], dsb[:])
            nc.gpsimd.collective_compute(
                "AllReduce", ALU.add, replica_groups=GROUPS,
                ins=[dloc.opt()], outs=[dred.opt()])
            ds = dpool.tile([128, ET * CH], DT.bfloat16, tag="dsA", bufs=2, name="dsA")
            nc.scalar.dma_start(ds[:], dred[:])
            for p in range(4):
                psl2 = slice(2 * p * CH, (2 * p + 2) * CH)
                nc.vector.tensor_tensor(x[ci][:, psl2], x[ci][:, psl2],
                                        ds[:, psl2], op=ALU.add)

        # persistent per-layer qkv tiles
        for l in range(L):
            # ---- layer weights to SBUF (merged single DMAs)
            wq = wpool.tile([128, ET * C], DT.bfloat16, tag="wq", name="wq")
            wk = wpool.tile([128, ET * C], DT.bfloat16, tag="wk", name="wk")
            wv = wpool.tile([128, ET * VW], DT.bfloat16, tag="wv", name="wv")
            wp = wpool.tile([128, 2 * E], DT.bfloat16, tag="wp", name="wp")
            bqk = wpool.tile([128, 4], DT.float32, tag="bqk", name="bqk")
            bvrow = wpool.tile([1, VW], DT.bfloat16, tag="bvrow", name="bvrow")
            nc.sync.dma_start(wq[:], inp[f"wqA{l}"][:])
            nc.sync.dma_start(wk[:], inp[f"wkA{l}"][:])
            nc.sync.dma_start(wv[:], inp[f"wvA{l}"][:])
            nc.sync.dma_start(wp[:], inp[f"wpA{l}"][:])
            nc.sync.dma_start(bqk[:], inp[f"bqk{l}"][:])
            nc.sync.dma_start(bvrow[:], inp[f"bvrow{l}"][:])
            w1 = wbig.tile([128, ET * F], DT.bfloat16, tag="w1", name="w1")
            w2 = wbig.tile([128, ET * E], DT.bfloat16, tag="w2", name="w2")
            bf2 = wpool.tile([128, 2 * ET], DT.float32, tag="bf2", name="bf2")
            nc.sync.dma_start(w1[:], inp[f"w1A{l}"][:])
            nc.sync.dma_start(w2[:], inp[f"w2A{l}"][:])
            nc.sync.dma_start(bf2[:], inp[f"bf2{l}"][:])

            qT = [[qkv.tile([128, CH], DT.bfloat16, tag=f"qT{ct}_{ci}", name=f"qT{ct}_{ci}")
                   for ci in range(NCH)] for ct in range(2)]
            kT = [[qkv.tile([128, CH], DT.bfloat16, tag=f"kT{ct}_{ci}", name=f"kT{ct}_{ci}")
                   for ci in range(NCH)] for ct in range(2)]
            v = [qkv.tile([128, VW], DT.bfloat16, tag=f"v{tt}", name=f"v{tt}")
                 for tt in range(8)]

            # ======== attention passes, one chunk at a time ========
            for ci in range(NCH):
                h = hpool.tile([128, ET * CH], DT.bfloat16, tag=f"h{ci}", name=f"h{ci}")
                ln_chunk(h, ci)

                # Q, K projections for this chunk
                for ct in range(2):
                    pq = ps_mm.tile([128, CH], DT.float32, tag="mm", name="mm")
                    for et in range(ET):
                        nc.tensor.matmul(pq[:], wq[:, et * C + ct * 128: et * C + (ct + 1) * 128],
                                         h[:, esl(et)], start=(et == 0), stop=(et == ET - 1))
                    nc.scalar.activation(qT[ct][ci][:], pq[:], AF.Identity,
                                         bias=bqk[:, ct:ct + 1])
                    pk = ps_mm.tile([128, CH], DT.float32, tag="mm", name="mm")
                    for et in range(ET):
                        nc.tensor.matmul(pk[:], wk[:, et * C + ct * 128: et * C + (ct + 1) * 128],
                                         h[:, esl(et)], start=(et == 0), stop=(et == ET - 1))
                    nc.scalar.activation(kT[ct][ci][:], pk[:], AF.Identity,
                                         bias=bqk[:, 2 + ct:3 + ct])

                # V (token-major, with ones column) for this chunk's 4 tiles
                for tt in range(4 * ci, 4 * ci + 4):
                    lsl = slice((tt % 4) * 128, (tt % 4) * 128 + 128)
                    pv = ps_mm.tile([128, VW], DT.float32, tag="mm", name="mm")
                    for et in range(ET):
                        nc.tensor.matmul(pv[:], h[:, et * CH + (tt % 4) * 128: et * CH + (tt % 4) * 128 + 128],
                                         wv[:, et * VW:(et + 1) * VW],
                                         start=(et == 0), stop=False)
                    nc.tensor.matmul(pv[:], ones128[:], bvrow[:], start=False, stop=True)
                    nc.vector.tensor_copy(v[tt][:], pv[:])

                # attention for this chunk; head-pairs packed on PE row groups
                o2 = [opool.tile([128, CH], DT.bfloat16, tag=f"o2_{hp}_{ci}", name=f"o2_{hp}_{ci}")
                      for hp in range(2)]
                nkt = 4 * ci + 4
                for hp in range(2):
                    ct = hp
                    opsA = ps_o.tile([65, CH], DT.float32, tag="o", name="oA")
                    opsB = ps_o.tile([65, CH], DT.float32, tag="o", name="oB")
                    for kt in range(nkt):
                        j = kt - 4 * ci
                        wexs = []
                        for sub in range(2):
                            psl = slice(64 * sub, 64 * sub + 64)
                            kcj, klo = kt // 4, (kt % 4) * 128
                            pw = ps_w.tile([128, CH], DT.float32, tag="w", name="w")
                            nc.tensor.matmul(pw[:], kT[ct][kcj][psl, klo:klo + 128],
                                             qT[ct][ci][psl, :], start=True, stop=True)
                            wex = wexpp.tile([128, CH], DT.bfloat16, tag="wx", bufs=6,
                                             name=f"wx{kt}_{sub}")
                            if j >= 0:
                                tmp = wexpp.tile([128, CH], DT.bfloat16, tag="wxt",
                                                 bufs=2, name="wxt")
                                nc.scalar.activation(tmp[:], pw[:], AF.Exp, scale=0.125)
                                nc.vector.tensor_tensor(wex[:], tmp[:], masks[j][:],
                                                        op=ALU.mult)
                            else:
                                nc.scalar.activation(wex[:], pw[:], AF.Exp, scale=0.125)
                            wexs.append(wex)
                        hA, hB = 2 * hp, 2 * hp + 1
                        nc.tensor.matmul(opsA[:], v[kt][:, hA * 65:(hA + 1) * 65], wexs[0][:],
                                         start=(kt == 0), stop=(kt == nkt - 1))
                        nc.tensor.matmul(opsB[:], v[kt][:, hB * 65:(hB + 1) * 65], wexs[1][:],
                                         start=(kt == 0), stop=(kt == nkt - 1))
                    for sub, ops in ((0, opsA), (1, opsB)):
                        psl = slice(64 * sub, 64 * sub + 64)
                        s32 = rowp.tile([1, CH], DT.float32, tag="s32", bufs=2, name="s32")
                        nc.scalar.activation(s32[:], ops[64:65, :], AF.Identity)
                        r32 = rowp.tile([1, CH], DT.float32, tag="r32", bufs=2, name="r32")
                        nc.vector.reciprocal(r32[:], s32[:])
                        r16 = rowp.tile([1, CH], DT.bfloat16, tag="r16", bufs=2, name="r16")
                        nc.scalar.activation(r16[:], r32[:], AF.Identity)
                        rb = ps_w.tile([64, CH], DT.float32, tag="w", name="rb")
                        nc.tensor.matmul(rb[:], ones64[:], r16[:], start=True, stop=True)
                        orw = rowp.tile([64, CH], DT.bfloat16, tag="orw", bufs=2, name="orw")
                        nc.scalar.activation(orw[:], ops[0:64, :], AF.Identity)
                        nc.vector.tensor_tensor(o2[hp][psl, :], orw[:], rb[:], op=ALU.mult)

                # proj for this chunk -> dsb -> AR -> residual
                dsb = dpool.tile([128, ET * CH], DT.bfloat16, tag="dsbA", bufs=2,
                                 name="dsbA")
                for et in range(ET):
                    pslE = slice(et * 128, (et + 1) * 128)
                    pd = ps_mm.tile([128, CH], DT.float32, tag="mm", name="mm")
                    nc.tensor.matmul(pd[:], wp[:, 0 * E + et * 128: 0 * E + (et + 1) * 128],
                                     o2[0][:], start=True, stop=False)
                    nc.tensor.matmul(pd[:], wp[:, 1 * E + et * 128: 1 * E + (et + 1) * 128],
                                     o2[1][:], start=False, stop=True)
                    nc.scalar.activation(dsb[:, esl(et)], pd[:], AF.Identity)
                ar_block(dsb, l, "a", ci)

            # ======== FFN passes, one chunk at a time ========
            for ci in range(NCH):
                h2 = hpool.tile([128, ET * CH], DT.bfloat16, tag=f"h{ci}", name=f"h2_{ci}")
                ln_chunk(h2, ci)
                ff = [dpool.tile([128, CH], DT.bfloat16, tag=f"ff{ft}", bufs=2,
                                 name=f"ff{ft}") for ft in range(ET)]
                for ft in range(ET):
                    pf = ps_mm.tile([128, CH], DT.float32, tag="mm", name="mm")
                    for et in range(ET):
                        nc.tensor.matmul(pf[:], w1[:, et * F + ft * 128: et * F + (ft + 1) * 128],
                                         h2[:, esl(et)], start=(et == 0), stop=(et == ET - 1))
                    nc.scalar.activation(ff[ft][:], pf[:], AF.Relu, bias=bf2[:, ft:ft + 1])
                dsb2 = dpool.tile([128, ET * CH], DT.bfloat16, tag="dsbA", bufs=2,
                                  name="dsb2")
                for et in range(ET):
                    pd = ps_mm.tile([128, CH], DT.float32, tag="mm", name="mm")
                    for ft in range(ET):
                        nc.tensor.matmul(pd[:], w2[:, ft * E + et * 128: ft * E + (et + 1) * 128],
                                         ff[ft][:], start=(ft == 0), stop=(ft == ET - 1))
                    nc.scalar.activation(dsb2[:, esl(et)], pd[:], AF.Identity,
                                         bias=bf2[:, ET + et:ET + et + 1])
                ar_block(dsb2, l, "f", ci)

        # ---- final LN + lm_head (ci-major so chunk 0 hides the last AR)
        for ci in range(NCH):
            hf = hpool.tile([128, ET * CH], DT.bfloat16, tag=f"h{ci}", name=f"hf{ci}")
            ln_chunk(hf, ci)
            sl = slice(ci * CH, (ci + 1) * CH)
            for vt in range(VCP // 128):
                vsl = slice(vt * 128, (vt + 1) * 128)
                emb = embp.tile([128, ET * 128], DT.bfloat16, tag="emb", name="emb")
                if vt % 2 == 0:
                    nc.sync.dma_start(emb[:], embA_d[vsl, :])
                else:
                    nc.scalar.dma_start(emb[:], embA_d[vsl, :])
                pool = ps_mm if vt % 2 == 0 else ps_w
                pl = pool.tile([128, CH], DT.float32,
                               tag=("mm" if vt % 2 == 0 else "w"), name="pl")
                for et in range(ET):
                    nc.tensor.matmul(pl[:], emb[:, et * 128:(et + 1) * 128],
                                     hf[:, esl(et)], start=(et == 0), stop=(et == ET - 1))
                lsb = lsbp.tile([128, CH], DT.bfloat16, tag="lsb", name="lsb")
                if vt % 2 == 0:
                    nc.scalar.activation(lsb[:], pl[:], AF.Identity)
                else:
                    nc.vector.tensor_copy(lsb[:], pl[:])
                nc.gpsimd.dma_start(out_d[vsl, sl], lsb[:])

    _split_sync_waits(nc)
    return nc


_NC = None


def _host_prep(inputs):
    """Fold LN params into weights, build per-core input maps."""
    f32 = np.float32

    def as_f32(vv):
        a = np.asarray(vv)
        return a if a.dtype in (np.int64, np.int32) else np.asarray(a, f32)

    g = {k: as_f32(vv) for k, vv in inputs.items()}
    idx = np.asarray(inputs["idx"])
    s = f32(E) ** -0.5

    mask_j = np.zeros((4, 128, CH), f32)
    q_idx = np.arange(CH)[None, :]
    k_idx = np.arange(128)[:, None]
    for j in range(4):
        mask_j[j] = (q_idx >= 128 * j + k_idx).astype(f32)

    def merge_et(mat, ncols):
        # [E, ncols] -> [128, ET*ncols] with et blocks along free axis
        return np.ascontiguousarray(
            mat.reshape(ET, 128, ncols).transpose(1, 0, 2).reshape(128, ET * ncols))

    per_layer = []
    for l in range(L):
        g1, b1v = g["ln1_g"][l], g["ln1_b"][l]
        g2, b2v = g["ln2_g"][l], g["ln2_b"][l]
        lay = []
        for r in range(TPD):
            csl = slice(C * r, C * (r + 1))
            fsl = slice(F * r, F * (r + 1))
            Wq_r, Wk_r, Wv_r = g["Wq"][l][csl], g["Wk"][l][csl], g["Wv"][l][csl]
            wvT = np.zeros((E, HC, 65), f32)
            bvrow = np.zeros((1, HC, 65), f32)
            for hh in range(HC):
                wslice = Wv_r[hh * HD:(hh + 1) * HD]          # [64, E]
                wvT[:, hh, :64] = (wslice * g1[None, :] * s).T
                bvrow[0, hh, :64] = wslice @ b1v * s
                bvrow[0, hh, 64] = 1.0
            bq = (Wq_r @ b1v * s).reshape(2, 128)
            bk = (Wk_r @ b1v * s).reshape(2, 128)
            bqk = np.stack([bq[0], bq[1], bk[0], bk[1]], axis=1)   # [128, 4]
            bfv = (g["W1"][l][fsl] @ b2v + g["b1"][l][fsl]).reshape(ET, 128)
            b2q = (g["b2"][l] / TPD).reshape(ET, 128)
            bf2 = np.concatenate([bfv.T, b2q.T], axis=1)           # [128, 16]
            d = {
                "wqA": merge_et((Wq_r * g1[None, :] * s).T, C),
                "wkA": merge_et((Wk_r * g1[None, :] * s).T, C),
                "wvA": merge_et(wvT.reshape(E, HC * 65), VW),
                "bqk": bqk,
                "bvrow": bvrow.reshape(1, VW),
                "wpA": np.ascontiguousarray(
                    (g["Wp"][l][:, csl] * s).T.reshape(2, 128, E)
                    .transpose(1, 0, 2).reshape(128, 2 * E)),
                "w1A": merge_et((g["W1"][l][fsl] * g2[None, :]).T, F),
                "w2A": np.ascontiguousarray(
                    g["W2"][l][:, fsl].T.reshape(ET, 128, E)
                    .transpose(1, 0, 2).reshape(128, ET * E)),
                "bf2": bf2,
            }
            lay.append(d)
        per_layer.append(lay)

    embA, hbias = [], []
    for r in range(TPD):
        vsl = slice(VC * r, VC * (r + 1))
        e = (g["tok_emb"][vsl] * g["lnf_g"][None, :]).T       # [E, 8000]
        ep = np.zeros((E, VCP), f32)
        ep[:, :VC] = e
        # [E, VCP] -> [VCP, E] tiled: embA[vt*128+p, et*128+vv] = ep[et*128+p, vt*128+vv]
        ea = ep.reshape(ET, 128, VCP // 128, 128).transpose(2, 1, 0, 3).reshape(VCP, E)
        embA.append(np.ascontiguousarray(ea))
        hbias.append(g["tok_emb"][vsl] @ g["lnf_b"] + g["head_b"][vsl])

    x0 = g["tok_emb"][idx] + g["pos_emb"][None, :T]           # [2, T, E]

    in_maps = []
    for c in range(NCORES):
        gb, r = c // TPD, c % TPD
        x0T = x0[gb].T                                         # [E, T]
        x0A = (x0T.reshape(ET, 128, NCH, CH).transpose(2, 1, 0, 3)
               .reshape(NCH, 128, ET * CH))
        m = {
            "x0A": np.ascontiguousarray(x0A).astype(bf16),
            "embA": embA[r].astype(bf16),
            "masks": mask_j.astype(bf16),
            "invE": np.full((128, 1), 1.0 / E, bf16),
            "ones128": np.ones((1, 128), bf16),
            "ones64": np.ones((1, 64), bf16),
        }
        for l in range(L):
            d = per_layer[l][r]
            for k, v_ in d.items():
                m[f"{k}{l}"] = v_.astype(f32) if k in ("bqk", "bf2") else v_.astype(bf16)
        in_maps.append(m)
    return in_maps, hbias


LAST_RESULT = None


def kernel(**inputs):
    global _NC, LAST_RESULT
    if _NC is None:
        _NC = _build_program()
    in_maps, hbias = _host_prep(inputs)
    import os
    trace = bool(os.environ.get("KBENCH_TRACE"))
    kw = {}
    if trace:
        import tempfile
        td = os.environ.get("KBENCH_TRACE_DIR")
        if td:
            os.makedirs(td, exist_ok=True)
        else:
            td = tempfile.mkdtemp(prefix="kbench_trace_")
        kw = dict(trace=True, tmpdir=td)
    res = run_bass_kernel_spmd(_NC, in_maps, list(range(NCORES)), **kw)
    LAST_RESULT = res
    B = 2
    logits = np.empty((B, T, V), np.float32)
    for c in range(NCORES):
        gb, r = c // TPD, c % TPD
        lt = np.asarray(res.results[c]["logitsT"][:VC, :], np.float32)  # [8000, T]
        logits[gb, :, VC * r:VC * (r + 1)] = lt.T + hbias[r][None, :]
    return logits
